# revision 12
# baseline (speedup 1.0000x reference)
"""Conv4d (Strang rearrange) Trainium2 kernel — banded-weight formulation.

Sharding: 8 cores = batch(4) x H-half(2). Each core computes the full
(D1=32, D2=32) conv for its batch sample and 16 output H rows.

Matmul formulation (per output D1-row u, one PSUM bank):
  Data lanes 0..79 = cin(4) x kh(2) x vin_l(10): vin_l spans the 10-wide
  input window 8b-1..8b+8 of D2 for the aligned output v-block b (8 wide).
  All 3 kv taps live INSIDE the banded stationary matrix
  W[(cin,kh,vl), (vo,cout)] (kv = vl - vo), so D2 needs no shifts and no
  cross-block halo matmuls. Lanes 80..127 are zero-weight padding that
  keeps the PE stationary tile at the full 128 rows — narrower loads
  serialize the 4 column-quadrant matmuls (measured 2x slower).
  M = 32 = vout_l(8) x cout(4); the 4 v-blocks land in the 4 PSUM
  partition quarters via tile_position=(0,32b), one concurrent wave of 4
  matmuls per round. Rounds: ku(<=3) x kw(2) -> 6 waves per u.
  Moving N = i(16) x j(32) = 512.

Moving data fp8e3 (e3m4), stationary bf16 (mixed matmul; rel err 0.013).
x HBM traffic: 10.2 MiB/core (1.25x vin-window duplication), output fp16
4.2 MiB/core. Lanes 80-127 of the 6 z slots are zero-filled by the first
six row loads (zpad); later rows leave stale fp8 there, nullified by the
zero weight rows.

Engine plan: sync + gpsimd interleave z-row loads (even/odd) with output
stores (even/odd u), scalar does w/bias/zpad DMA + psum->sbuf activations
(Identity + bias), tensor runs the matmul pipeline.
PSUM ring 6, z-row ring 6, out ring 4.
"""

from contextlib import ExitStack

import ml_dtypes
import numpy as np

import concourse.bass as bass
from concourse import bacc, mybir
from concourse.bass_utils import run_bass_kernel_spmd

F16 = mybir.dt.float16
BF16 = mybir.dt.bfloat16
F32 = mybir.dt.float32
F8E3 = mybir.dt.float8e3

B, CIN, COUT = 4, 4, 4
D1, D2, H, W = 32, 32, 64, 64
U = D1  # 32 output rows per core
V = D2
IL, J = 16, 32  # per-core output i (H/2 per half), j
NB = 4  # v-blocks of 8
VL = 10  # vin lanes per block (8 + 2 halo)
KD = CIN * 2 * VL  # 80 data lanes
K = 128
NCORES = 8
NZ, NPS, NOUT = 6, 6, 4

ROUNDS = [(ku, kw) for ku in range(3) for kw in range(2)]


def _host_weights(w, b):
    w = np.asarray(w, np.float32)
    wbd = np.zeros((K, 6, 32), np.float32)
    for ri, (ku, kw) in enumerate(ROUNDS):
        for kh in range(2):
            for vl in range(VL):
                for vo in range(8):
                    kv = vl - vo
                    if 0 <= kv <= 2:
                        # lane p = (cin*2+kh)*10+vl ; col m = vo*4+cout
                        wbd[kh * VL + vl : KD : 2 * VL, ri, vo * 4 : vo * 4 + 4] = w[
                            :, :, ku, kv, kh, kw
                        ].T
    bias = np.tile(np.asarray(b, np.float32), 32).reshape(128, 1)
    return wbd.astype(ml_dtypes.bfloat16), bias


def _host_shard(x):
    x = np.asarray(x, np.float32)
    shards = []
    vidx = (8 * np.arange(NB)[:, None] + np.arange(VL)[None, :]).reshape(-1)  # 0..33
    for core in range(NCORES):
        bb, hh = divmod(core, 2)
        xc = x[bb, :, :, :, 32 * hh : 32 * hh + 32, :]  # [cin, r, v, h, w]
        xp = np.pad(xc, ((0, 0), (0, 0), (1, 1), (0, 0), (0, 0)))  # v -> 34
        arr = xp[:, :, vidx]  # [cin, r, b*vl, h, w]
        arr = arr.reshape(CIN, U, NB, VL, IL, 2, J, 2)
        # dims: cin0 r1 b2 vl3 i4 kh5 j6 kw7 -> [r, cin, kh, vl, kw, b, i, j]
        arr = arr.transpose(1, 0, 5, 3, 7, 2, 4, 6)
        arr = np.ascontiguousarray(arr).astype(ml_dtypes.float8_e3m4)
        shards.append(arr.reshape(U, KD, 2, NB, IL, J))
    return shards


def _build_program():
    nc = bacc.Bacc("TRN2", target_bir_lowering=False, debug=False)
    zs = nc.dram_tensor("zs", [U, KD, 2, NB, IL, J], F8E3, kind="ExternalInput").ap()
    zpad = nc.dram_tensor("zpad", [K - KD, 2, NB, IL, J], F8E3, kind="ExternalInput").ap()
    wbd = nc.dram_tensor("wbd", [K, 6, 32], BF16, kind="ExternalInput").ap()
    bias = nc.dram_tensor("bias", [128, 1], F32, kind="ExternalInput").ap()
    ys = nc.dram_tensor("ys", [U, 128, IL, J], F16, kind="ExternalOutput").ap()

    def ztot(r):
        return 32 if r < NZ else 16

    with ExitStack() as ctx:
        zt = [ctx.enter_context(nc.sbuf_tensor(f"z{i}", [K, 2, NB, IL, J], F8E3)) for i in range(NZ)]
        wt = ctx.enter_context(nc.sbuf_tensor("wt", [K, 6, 32], BF16))
        bt = ctx.enter_context(nc.sbuf_tensor("bt", [128, 1], F32))
        ot = [ctx.enter_context(nc.sbuf_tensor(f"ot{i}", [128, IL, J], F16)) for i in range(NOUT)]
        ps = [ctx.enter_context(nc.psum_tensor(f"ps{i}", [128, IL, J], F32)) for i in range(NPS)]
        sem_z = [ctx.enter_context(nc.semaphore(f"sem_z{r}")) for r in range(U)]
        sem_w = ctx.enter_context(nc.semaphore("sem_w"))
        sem_b = ctx.enter_context(nc.semaphore("sem_b"))
        sem_mm = ctx.enter_context(nc.semaphore("sem_mm"))
        sem_act = ctx.enter_context(nc.semaphore("sem_act"))
        sem_os = [ctx.enter_context(nc.semaphore(f"sem_o{i}")) for i in range(NOUT)]
        blk_ctx = nc.Block()
        block = blk_ctx.__enter__()

        def _io_engine(eng, par):
            # z rows (par)ity preload, then per-u: output store + next z row
            for r in range(par, NZ, 2):
                eng.dma_start(zt[r][:KD], zs[r]).then_inc(sem_z[r], 16)
            for u in range(par, U, 2):
                eng.wait_ge(sem_act, u + 1)
                eng.dma_start(ys[u], ot[u % NOUT][:]).then_inc(
                    sem_os[u % NOUT], 16
                )
                r = u + NZ
                if r < U:
                    eng.wait_ge(sem_mm, r - 4)
                    eng.dma_start(zt[r % NZ][:KD], zs[r]).then_inc(sem_z[r], 16)

        @block.sync
        def _(sync):
            _io_engine(sync, 0)
            for s in sem_os:
                sync.wait_ge(s, 16 * (U // NOUT))

        @block.gpsimd
        def _(gpsimd):
            _io_engine(gpsimd, 1)

        @block.scalar
        def _(scalar):
            # zero-fill pad lanes of the first-generation slots; later rows
            # leave stale fp8 there, nullified by zero weight rows
            for i in range(2):
                scalar.dma_start(zt[i][KD:], zpad[:]).then_inc(sem_z[i], 16)
            scalar.dma_start(wt[:], wbd[:]).then_inc(sem_w, 16)
            scalar.dma_start(bt[:], bias[:]).then_inc(sem_b, 16)
            for i in range(2, NZ):
                scalar.dma_start(zt[i][KD:], zpad[:]).then_inc(sem_z[i], 16)
            for u in range(U):
                scalar.wait_ge(sem_mm, u + 1)
                if u == 0:
                    scalar.wait_ge(sem_b, 16)
                if u >= NOUT:
                    scalar.wait_ge(sem_os[u % NOUT], 16 * (u // NOUT))
                nc.scalar.activation(
                    ot[u % NOUT][:],
                    ps[u % NPS][:],
                    mybir.ActivationFunctionType.Identity,
                    bias=bt[:],
                ).then_inc(sem_act)

        @block.tensor
        def _(tensor):
            for u in range(U):
                if u == 0:
                    tensor.wait_ge(sem_w, 16)
                    tensor.wait_ge(sem_z[0], ztot(0))
                    tensor.wait_ge(sem_z[1], ztot(1))
                elif u + 1 < U:
                    tensor.wait_ge(sem_z[u + 1], ztot(u + 1))
                if u >= NPS:
                    tensor.wait_ge(sem_act, u - NPS + 1)
                psg = ps[u % NPS]
                rounds = [(ku, kw) for ku in range(3) if 0 <= u + ku - 1 < U for kw in range(2)]
                last = None
                for idx, (ku, kw) in enumerate(rounds):
                    ri = ku * 2 + kw
                    slot = (u + ku - 1) % NZ
                    for b4 in range(NB):
                        last = nc.tensor.matmul(
                            psg[32 * b4 : 32 * b4 + 32, :, :],
                            wt[:, ri, :],
                            zt[slot][:, kw, b4, :, :],
                            start=(idx == 0),
                            stop=(idx == len(rounds) - 1),
                            skip_group_check=True,
                            tile_position=(0, 32 * b4),
                        )
                last.then_inc(sem_mm)

        blk_ctx.__exit__(None, None, None)

    nc.compile()
    return nc


def _unshard(results):
    y = np.empty((B, COUT, D1, D2, H // 2, W // 2), np.float32)
    for core in range(NCORES):
        bb, hh = divmod(core, 2)
        arr = results[core]["ys"].astype(np.float32)
        arr = arr.reshape(U, NB, 8, COUT, IL, J)
        arr = arr.transpose(3, 0, 1, 2, 4, 5).reshape(COUT, U, V, IL, J)
        y[bb, :, :, :, 16 * hh : 16 * hh + 16, :] = arr
    return y


TRACE = False
LAST_RESULT = [None]

_ZPAD = np.zeros((K - KD, 2, NB, IL, J), ml_dtypes.float8_e3m4)


def kernel(x, w, b, _cache={}):
    if "nc" not in _cache:
        _cache["nc"] = _build_program()
    nc = _cache["nc"]
    wbd, bias = _host_weights(w, b)
    in_maps = [
        {"zs": zs, "zpad": _ZPAD, "wbd": wbd, "bias": bias} for zs in _host_shard(x)
    ]
    res = run_bass_kernel_spmd(nc, in_maps, list(range(NCORES)), trace=TRACE)
    LAST_RESULT[0] = res
    return _unshard(res.results)


# revision 13
# speedup vs baseline: 1.0977x; 1.0977x over previous
"""Conv4d (Strang rearrange) Trainium2 kernel — banded-weight formulation.

Sharding: 8 cores = batch(4) x H-half(2). Each core computes the full
(D1=32, D2=32) conv for its batch sample and 16 output H rows.

Matmul formulation (per output D1-row u, one PSUM bank):
  Data lanes 0..79 = cin(4) x kh(2) x vin_l(10): vin_l spans the 10-wide
  input window 8b-1..8b+8 of D2 for the aligned output v-block b (8 wide).
  All 3 kv taps live INSIDE the banded stationary matrix
  W[(cin,kh,vl), (vo,cout)] (kv = vl - vo), so D2 needs no shifts and no
  cross-block halo matmuls. Lanes 80..127 are zero-weight padding that
  keeps the PE stationary tile at the full 128 rows — narrower loads
  serialize the 4 column-quadrant matmuls (measured 2x slower).
  M = 32 = vout_l(8) x cout(4); the 4 v-blocks land in the 4 PSUM
  partition quarters via tile_position=(0,32b), one concurrent wave of 4
  matmuls per round. Rounds: ku(<=3) x kw(2) -> 6 waves per u.
  Moving N = i(16) x j(32) = 512.

Moving data fp8e3 (e3m4), stationary bf16 (mixed matmul; rel err 0.013).
x HBM traffic: 10.2 MiB/core (1.25x vin-window duplication), output fp16
4.2 MiB/core. Lanes 80-127 of the 6 z slots are zero-filled by the first
six row loads (zpad); later rows leave stale fp8 there, nullified by the
zero weight rows.

Engine plan: sync + gpsimd interleave z-row loads (even/odd) with output
stores (even/odd u), scalar does w/bias/zpad DMA + psum->sbuf activations
(Identity + bias), tensor runs the matmul pipeline.
PSUM ring 6, z-row ring 6, out ring 4.
"""

from contextlib import ExitStack

import ml_dtypes
import numpy as np

import concourse.bass as bass
from concourse import bacc, mybir
from concourse.bass_utils import run_bass_kernel_spmd

F16 = mybir.dt.float16
BF16 = mybir.dt.bfloat16
F32 = mybir.dt.float32
F8E3 = mybir.dt.float8e3

B, CIN, COUT = 4, 4, 4
D1, D2, H, W = 32, 32, 64, 64
U = D1  # 32 output rows per core
V = D2
IL, J = 16, 32  # per-core output i (H/2 per half), j
NB = 4  # v-blocks of 8
VL = 10  # vin lanes per block (8 + 2 halo)
KD = CIN * 2 * VL  # 80 data lanes
K = 128
NCORES = 8
NZ, NPS, NOUT = 8, 8, 4

ROUNDS = [(ku, kw) for ku in range(3) for kw in range(2)]


def _host_weights(w, b):
    w = np.asarray(w, np.float32)
    wbd = np.zeros((K, 6, 32), np.float32)
    for ri, (ku, kw) in enumerate(ROUNDS):
        for kh in range(2):
            for vl in range(VL):
                for vo in range(8):
                    kv = vl - vo
                    if 0 <= kv <= 2:
                        # lane p = (cin*2+kh)*10+vl ; col m = vo*4+cout
                        wbd[kh * VL + vl : KD : 2 * VL, ri, vo * 4 : vo * 4 + 4] = w[
                            :, :, ku, kv, kh, kw
                        ].T
    bias = np.tile(np.asarray(b, np.float32), 32).reshape(128, 1)
    return wbd.astype(ml_dtypes.bfloat16), bias


def _host_shard(x):
    x = np.asarray(x, np.float32)
    shards = []
    vidx = (8 * np.arange(NB)[:, None] + np.arange(VL)[None, :]).reshape(-1)  # 0..33
    for core in range(NCORES):
        bb, hh = divmod(core, 2)
        xc = x[bb, :, :, :, 32 * hh : 32 * hh + 32, :]  # [cin, r, v, h, w]
        xp = np.pad(xc, ((0, 0), (0, 0), (1, 1), (0, 0), (0, 0)))  # v -> 34
        arr = xp[:, :, vidx]  # [cin, r, b*vl, h, w]
        arr = arr.reshape(CIN, U, NB, VL, IL, 2, J, 2)
        # dims: cin0 r1 b2 vl3 i4 kh5 j6 kw7 -> [r, cin, kh, vl, kw, b, i, j]
        arr = arr.transpose(1, 0, 5, 3, 7, 2, 4, 6)
        arr = np.ascontiguousarray(arr).astype(ml_dtypes.float8_e3m4)
        shards.append(arr.reshape(U, KD, 2, NB, IL, J))
    return shards


def _build_program():
    nc = bacc.Bacc("TRN2", target_bir_lowering=False, debug=False)
    zs = nc.dram_tensor("zs", [U, KD, 2, NB, IL, J], F8E3, kind="ExternalInput").ap()
    zpad = nc.dram_tensor("zpad", [K - KD, 2, NB, IL, J], F8E3, kind="ExternalInput").ap()
    wbd = nc.dram_tensor("wbd", [K, 6, 32], BF16, kind="ExternalInput").ap()
    bias = nc.dram_tensor("bias", [128, 1], F32, kind="ExternalInput").ap()
    ys = nc.dram_tensor("ys", [U, 128, IL, J], F16, kind="ExternalOutput").ap()

    def ztot(r):
        return 32 if r < NZ else 16

    with ExitStack() as ctx:
        zt = [ctx.enter_context(nc.sbuf_tensor(f"z{i}", [K, 2, NB, IL, J], F8E3)) for i in range(NZ)]
        wt = ctx.enter_context(nc.sbuf_tensor("wt", [K, 6, 32], BF16))
        bt = ctx.enter_context(nc.sbuf_tensor("bt", [128, 1], F32))
        ot = [ctx.enter_context(nc.sbuf_tensor(f"ot{i}", [128, IL, J], F16)) for i in range(NOUT)]
        ps = [ctx.enter_context(nc.psum_tensor(f"ps{i}", [128, IL, J], F32)) for i in range(NPS)]
        sem_z = [ctx.enter_context(nc.semaphore(f"sem_z{r}")) for r in range(U)]
        sem_w = ctx.enter_context(nc.semaphore("sem_w"))
        sem_b = ctx.enter_context(nc.semaphore("sem_b"))
        sem_mm = ctx.enter_context(nc.semaphore("sem_mm"))
        sem_act = ctx.enter_context(nc.semaphore("sem_act"))
        sem_os = [ctx.enter_context(nc.semaphore(f"sem_o{i}")) for i in range(NOUT)]
        blk_ctx = nc.Block()
        block = blk_ctx.__enter__()

        def _io_engine(eng, par):
            # z rows (par)ity preload, then per-u: output store + next z row
            for r in range(par, NZ, 2):
                eng.dma_start(zt[r][:KD], zs[r]).then_inc(sem_z[r], 16)
            for u in range(par, U, 2):
                eng.wait_ge(sem_act, u + 1)
                eng.dma_start(ys[u], ot[u % NOUT][:]).then_inc(
                    sem_os[u % NOUT], 16
                )
                r = u + NZ
                if r < U:
                    eng.wait_ge(sem_mm, r - NZ + 2)
                    eng.dma_start(zt[r % NZ][:KD], zs[r]).then_inc(sem_z[r], 16)

        @block.sync
        def _(sync):
            _io_engine(sync, 0)
            for s in sem_os:
                sync.wait_ge(s, 16 * (U // NOUT))

        @block.gpsimd
        def _(gpsimd):
            _io_engine(gpsimd, 1)

        @block.scalar
        def _(scalar):
            # zero-fill pad lanes of the first-generation slots; later rows
            # leave stale fp8 there, nullified by zero weight rows
            for i in range(2):
                scalar.dma_start(zt[i][KD:], zpad[:]).then_inc(sem_z[i], 16)
            scalar.dma_start(wt[:], wbd[:]).then_inc(sem_w, 16)
            scalar.dma_start(bt[:], bias[:]).then_inc(sem_b, 16)
            for i in range(2, NZ):
                scalar.dma_start(zt[i][KD:], zpad[:]).then_inc(sem_z[i], 16)
            for u in range(U):
                scalar.wait_ge(sem_mm, u + 1)
                if u == 0:
                    scalar.wait_ge(sem_b, 16)
                if u >= NOUT:
                    scalar.wait_ge(sem_os[u % NOUT], 16 * (u // NOUT))
                nc.scalar.activation(
                    ot[u % NOUT][:],
                    ps[u % NPS][:],
                    mybir.ActivationFunctionType.Identity,
                    bias=bt[:],
                ).then_inc(sem_act)

        @block.tensor
        def _(tensor):
            for u in range(U):
                if u == 0:
                    tensor.wait_ge(sem_w, 16)
                    tensor.wait_ge(sem_z[0], ztot(0))
                    tensor.wait_ge(sem_z[1], ztot(1))
                elif u + 1 < U:
                    tensor.wait_ge(sem_z[u + 1], ztot(u + 1))
                if u >= NPS:
                    tensor.wait_ge(sem_act, u - NPS + 1)
                psg = ps[u % NPS]
                rounds = [(ku, kw) for ku in range(3) if 0 <= u + ku - 1 < U for kw in range(2)]
                last = None
                for idx, (ku, kw) in enumerate(rounds):
                    ri = ku * 2 + kw
                    slot = (u + ku - 1) % NZ
                    for b4 in range(NB):
                        last = nc.tensor.matmul(
                            psg[32 * b4 : 32 * b4 + 32, :, :],
                            wt[:, ri, :],
                            zt[slot][:, kw, b4, :, :],
                            start=(idx == 0),
                            stop=(idx == len(rounds) - 1),
                            skip_group_check=True,
                            tile_position=(0, 32 * b4),
                        )
                last.then_inc(sem_mm)

        blk_ctx.__exit__(None, None, None)

    nc.compile()
    return nc


def _unshard(results):
    y = np.empty((B, COUT, D1, D2, H // 2, W // 2), np.float32)
    for core in range(NCORES):
        bb, hh = divmod(core, 2)
        arr = results[core]["ys"].astype(np.float32)
        arr = arr.reshape(U, NB, 8, COUT, IL, J)
        arr = arr.transpose(3, 0, 1, 2, 4, 5).reshape(COUT, U, V, IL, J)
        y[bb, :, :, :, 16 * hh : 16 * hh + 16, :] = arr
    return y


TRACE = False
LAST_RESULT = [None]

_ZPAD = np.zeros((K - KD, 2, NB, IL, J), ml_dtypes.float8_e3m4)


def kernel(x, w, b, _cache={}):
    if "nc" not in _cache:
        _cache["nc"] = _build_program()
    nc = _cache["nc"]
    wbd, bias = _host_weights(w, b)
    in_maps = [
        {"zs": zs, "zpad": _ZPAD, "wbd": wbd, "bias": bias} for zs in _host_shard(x)
    ]
    res = run_bass_kernel_spmd(nc, in_maps, list(range(NCORES)), trace=TRACE)
    LAST_RESULT[0] = res
    return _unshard(res.results)


# revision 14
# speedup vs baseline: 1.1215x; 1.0216x over previous
"""Conv4d (Strang rearrange) Trainium2 kernel — banded-weight formulation.

Sharding: 8 cores = batch(4) x H-half(2). Each core computes the full
(D1=32, D2=32) conv for its batch sample and 16 output H rows.

Matmul formulation (per output D1-row u, one PSUM bank):
  Data lanes 0..79 = cin(4) x kh(2) x vin_l(10): vin_l spans the 10-wide
  input window 8b-1..8b+8 of D2 for the aligned output v-block b (8 wide).
  All 3 kv taps live INSIDE the banded stationary matrix
  W[(cin,kh,vl), (vo,cout)] (kv = vl - vo), so D2 needs no shifts and no
  cross-block halo matmuls. Lanes 80..127 are zero-weight padding that
  keeps the PE stationary tile at the full 128 rows — narrower loads
  serialize the 4 column-quadrant matmuls (measured 2x slower).
  M = 32 = vout_l(8) x cout(4); the 4 v-blocks land in the 4 PSUM
  partition quarters via tile_position=(0,32b), one concurrent wave of 4
  matmuls per round. Rounds: ku(<=3) x kw(2) -> 6 waves per u.
  Moving N = i(16) x j(32) = 512.

Moving data fp8e3 (e3m4), stationary bf16 (mixed matmul; rel err 0.013).
x HBM traffic: 10.2 MiB/core (1.25x vin-window duplication), output fp16
4.2 MiB/core. Lanes 80-127 of the 6 z slots are zero-filled by the first
six row loads (zpad); later rows leave stale fp8 there, nullified by the
zero weight rows.

Engine plan: sync + gpsimd interleave z-row loads (even/odd) with output
stores (even/odd u), scalar does w/bias/zpad DMA + psum->sbuf activations
(Identity + bias), tensor runs the matmul pipeline.
PSUM ring 6, z-row ring 6, out ring 4.
"""

from contextlib import ExitStack

import ml_dtypes
import numpy as np

import concourse.bass as bass
from concourse import bacc, mybir
from concourse.bass_utils import run_bass_kernel_spmd

F16 = mybir.dt.float16
BF16 = mybir.dt.bfloat16
F32 = mybir.dt.float32
F8E3 = mybir.dt.float8e3

B, CIN, COUT = 4, 4, 4
D1, D2, H, W = 32, 32, 64, 64
U = D1  # 32 output rows per core
V = D2
IL, J = 16, 32  # per-core output i (H/2 per half), j
NB = 4  # v-blocks of 8
VL = 10  # vin lanes per block (8 + 2 halo)
KD = CIN * 2 * VL  # 80 data lanes
K = 128
NCORES = 8
NZ, NPS, NOUT = 8, 8, 4

ROUNDS = [(ku, kw) for ku in range(3) for kw in range(2)]


def _host_weights(w, b):
    w = np.asarray(w, np.float32)
    wbd = np.zeros((K, 6, 32), np.float32)
    for ri, (ku, kw) in enumerate(ROUNDS):
        for kh in range(2):
            for vl in range(VL):
                for vo in range(8):
                    kv = vl - vo
                    if 0 <= kv <= 2:
                        # lane p = (cin*2+kh)*10+vl ; col m = vo*4+cout
                        wbd[kh * VL + vl : KD : 2 * VL, ri, vo * 4 : vo * 4 + 4] = w[
                            :, :, ku, kv, kh, kw
                        ].T
    bias = np.tile(np.asarray(b, np.float32), 32).reshape(128, 1)
    return wbd.astype(ml_dtypes.bfloat16), bias


def _host_shard(x):
    x = np.asarray(x, np.float32)
    shards = []
    vidx = (8 * np.arange(NB)[:, None] + np.arange(VL)[None, :]).reshape(-1)  # 0..33
    for core in range(NCORES):
        bb, hh = divmod(core, 2)
        xc = x[bb, :, :, :, 32 * hh : 32 * hh + 32, :]  # [cin, r, v, h, w]
        xp = np.pad(xc, ((0, 0), (0, 0), (1, 1), (0, 0), (0, 0)))  # v -> 34
        arr = xp[:, :, vidx]  # [cin, r, b*vl, h, w]
        arr = arr.reshape(CIN, U, NB, VL, IL, 2, J, 2)
        # dims: cin0 r1 b2 vl3 i4 kh5 j6 kw7 -> [r, cin, kh, vl, kw, b, i, j]
        arr = arr.transpose(1, 0, 5, 3, 7, 2, 4, 6)
        arr = np.ascontiguousarray(arr).astype(ml_dtypes.float8_e3m4)
        shards.append(arr.reshape(U, KD, 2, NB, IL, J))
    return shards


def _build_program():
    nc = bacc.Bacc("TRN2", target_bir_lowering=False, debug=False)
    zs = nc.dram_tensor("zs", [U, KD, 2, NB, IL, J], F8E3, kind="ExternalInput").ap()
    zpad = nc.dram_tensor("zpad", [K - KD, 2, NB, IL, J], F8E3, kind="ExternalInput").ap()
    wbd = nc.dram_tensor("wbd", [K, 6, 32], BF16, kind="ExternalInput").ap()
    bias = nc.dram_tensor("bias", [128, 1], F32, kind="ExternalInput").ap()
    ys = nc.dram_tensor("ys", [U, 128, IL, J], F16, kind="ExternalOutput").ap()

    def ztot(r):
        return 32 if r < NZ else 16

    with ExitStack() as ctx:
        zt = [ctx.enter_context(nc.sbuf_tensor(f"z{i}", [K, 2, NB, IL, J], F8E3)) for i in range(NZ)]
        wt = ctx.enter_context(nc.sbuf_tensor("wt", [K, 6, 32], BF16))
        bt = ctx.enter_context(nc.sbuf_tensor("bt", [128, 1], F32))
        ot = [ctx.enter_context(nc.sbuf_tensor(f"ot{i}", [128, IL, J], F16)) for i in range(NOUT)]
        ps = [ctx.enter_context(nc.psum_tensor(f"ps{i}", [128, IL, J], F32)) for i in range(NPS)]
        sem_z = [ctx.enter_context(nc.semaphore(f"sem_z{r}")) for r in range(U)]
        sem_w = ctx.enter_context(nc.semaphore("sem_w"))
        sem_b = ctx.enter_context(nc.semaphore("sem_b"))
        sem_mm = ctx.enter_context(nc.semaphore("sem_mm"))
        sem_act = ctx.enter_context(nc.semaphore("sem_act"))
        sem_os = [ctx.enter_context(nc.semaphore(f"sem_o{i}")) for i in range(NOUT)]
        blk_ctx = nc.Block()
        block = blk_ctx.__enter__()

        def _io_engine(eng, par, skip_stores=()):
            # row `par` feeds u=0; everything else is staged behind sem_w so
            # the first groups' deps aren't diluted on the shared DMA device
            eng.dma_start(zt[par][:KD], zs[par]).then_inc(sem_z[par], 16)
            eng.wait_ge(sem_w, 16)
            for r in (2 + par, 4 + par):
                eng.dma_start(zt[r][:KD], zs[r]).then_inc(sem_z[r], 16)
            for u in range(par, U, 2):
                r = u + 6
                if r < U:
                    if r >= NZ:
                        eng.wait_ge(sem_mm, r - 6)
                    eng.dma_start(zt[r % NZ][:KD], zs[r]).then_inc(sem_z[r], 16)
                if u not in skip_stores:
                    eng.wait_ge(sem_act, u + 1)
                    eng.dma_start(ys[u], ot[u % NOUT][:]).then_inc(
                        sem_os[u % NOUT], 16
                    )

        @block.sync
        def _(sync):
            _io_engine(sync, 0)
            # last odd stores run here so gpsimd's slow SWDGE drain overlaps
            for u in (29, 31):
                sync.wait_ge(sem_act, u + 1)
                sync.dma_start(ys[u], ot[u % NOUT][:]).then_inc(
                    sem_os[u % NOUT], 16
                )
            for s in sem_os:
                sync.wait_ge(s, 16 * (U // NOUT))

        @block.gpsimd
        def _(gpsimd):
            _io_engine(gpsimd, 1, skip_stores=(29, 31))

        @block.scalar
        def _(scalar):
            # zero-fill pad lanes of the first-generation slots; later rows
            # leave stale fp8 there, nullified by zero weight rows
            for i in range(2):
                scalar.dma_start(zt[i][KD:], zpad[:]).then_inc(sem_z[i], 16)
            scalar.dma_start(wt[:], wbd[:]).then_inc(sem_w, 16)
            scalar.dma_start(bt[:], bias[:]).then_inc(sem_b, 16)
            for i in range(2, NZ):
                scalar.dma_start(zt[i][KD:], zpad[:]).then_inc(sem_z[i], 16)
            for u in range(U):
                scalar.wait_ge(sem_mm, u + 1)
                if u == 0:
                    scalar.wait_ge(sem_b, 16)
                if u >= NOUT:
                    scalar.wait_ge(sem_os[u % NOUT], 16 * (u // NOUT))
                nc.scalar.activation(
                    ot[u % NOUT][:],
                    ps[u % NPS][:],
                    mybir.ActivationFunctionType.Identity,
                    bias=bt[:],
                ).then_inc(sem_act)

        @block.tensor
        def _(tensor):
            for u in range(U):
                if u == 0:
                    tensor.wait_ge(sem_w, 16)
                    tensor.wait_ge(sem_z[0], ztot(0))
                    tensor.wait_ge(sem_z[1], ztot(1))
                elif u + 1 < U:
                    tensor.wait_ge(sem_z[u + 1], ztot(u + 1))
                if u >= NPS:
                    tensor.wait_ge(sem_act, u - NPS + 1)
                psg = ps[u % NPS]
                rounds = [(ku, kw) for ku in range(3) if 0 <= u + ku - 1 < U for kw in range(2)]
                last = None
                for idx, (ku, kw) in enumerate(rounds):
                    ri = ku * 2 + kw
                    slot = (u + ku - 1) % NZ
                    for b4 in range(NB):
                        last = nc.tensor.matmul(
                            psg[32 * b4 : 32 * b4 + 32, :, :],
                            wt[:, ri, :],
                            zt[slot][:, kw, b4, :, :],
                            start=(idx == 0),
                            stop=(idx == len(rounds) - 1),
                            skip_group_check=True,
                            tile_position=(0, 32 * b4),
                        )
                last.then_inc(sem_mm)

        blk_ctx.__exit__(None, None, None)

    nc.compile()
    return nc


def _unshard(results):
    y = np.empty((B, COUT, D1, D2, H // 2, W // 2), np.float32)
    for core in range(NCORES):
        bb, hh = divmod(core, 2)
        arr = results[core]["ys"].astype(np.float32)
        arr = arr.reshape(U, NB, 8, COUT, IL, J)
        arr = arr.transpose(3, 0, 1, 2, 4, 5).reshape(COUT, U, V, IL, J)
        y[bb, :, :, :, 16 * hh : 16 * hh + 16, :] = arr
    return y


TRACE = False
LAST_RESULT = [None]

_ZPAD = np.zeros((K - KD, 2, NB, IL, J), ml_dtypes.float8_e3m4)


def kernel(x, w, b, _cache={}):
    if "nc" not in _cache:
        _cache["nc"] = _build_program()
    nc = _cache["nc"]
    wbd, bias = _host_weights(w, b)
    in_maps = [
        {"zs": zs, "zpad": _ZPAD, "wbd": wbd, "bias": bias} for zs in _host_shard(x)
    ]
    res = run_bass_kernel_spmd(nc, in_maps, list(range(NCORES)), trace=TRACE)
    LAST_RESULT[0] = res
    return _unshard(res.results)


# revision 15
# speedup vs baseline: 1.1676x; 1.0411x over previous
"""Conv4d (Strang rearrange) Trainium2 kernel — banded-weight formulation.

Sharding: 8 cores = batch(4) x H-half(2). Each core computes the full
(D1=32, D2=32) conv for its batch sample and 16 output H rows.

Matmul formulation (per output D1-row u, one PSUM bank):
  Data lanes 0..79 = cin(4) x kh(2) x vin_l(10): vin_l spans the 10-wide
  input window 8b-1..8b+8 of D2 for the aligned output v-block b (8 wide).
  All 3 kv taps live INSIDE the banded stationary matrix
  W[(cin,kh,vl), (vo,cout)] (kv = vl - vo), so D2 needs no shifts and no
  cross-block halo matmuls. Lanes 80..127 are zero-weight padding that
  keeps the PE stationary tile at the full 128 rows — narrower loads
  serialize the 4 column-quadrant matmuls (measured 2x slower).
  M = 32 = vout_l(8) x cout(4); the 4 v-blocks land in the 4 PSUM
  partition quarters via tile_position=(0,32b), one concurrent wave of 4
  matmuls per round. Rounds: ku(<=3) x kw(2) -> 6 waves per u.
  Moving N = i(16) x j(32) = 512.

Moving data fp8e3 (e3m4), stationary bf16 (mixed matmul; rel err 0.013).
x HBM traffic: 10.2 MiB/core (1.25x vin-window duplication), output fp16
4.2 MiB/core. Lanes 80-127 of the 6 z slots are zero-filled by the first
six row loads (zpad); later rows leave stale fp8 there, nullified by the
zero weight rows.

Engine plan: sync + gpsimd interleave z-row loads (even/odd) with output
stores (even/odd u), scalar does w/bias/zpad DMA + psum->sbuf activations
(Identity + bias), tensor runs the matmul pipeline.
PSUM ring 6, z-row ring 6, out ring 4.
"""

from contextlib import ExitStack

import ml_dtypes
import numpy as np

import concourse.bass as bass
from concourse import bacc, mybir
from concourse.bass_utils import run_bass_kernel_spmd

F16 = mybir.dt.float16
BF16 = mybir.dt.bfloat16
F32 = mybir.dt.float32
F8E3 = mybir.dt.float8e3

B, CIN, COUT = 4, 4, 4
D1, D2, H, W = 32, 32, 64, 64
U = D1  # 32 output rows per core
V = D2
IL, J = 16, 32  # per-core output i (H/2 per half), j
NB = 4  # v-blocks of 8
VL = 10  # vin lanes per block (8 + 2 halo)
KD = CIN * 2 * VL  # 80 data lanes
K = 128
NCORES = 8
NZ, NPS, NOUT = 8, 8, 4

ROUNDS = [(ku, kw) for ku in range(3) for kw in range(2)]


def _host_weights(w, b):
    w = np.asarray(w, np.float32)
    wbd = np.zeros((K, 6, 32), np.float32)
    for ri, (ku, kw) in enumerate(ROUNDS):
        for kh in range(2):
            for vl in range(VL):
                for vo in range(8):
                    kv = vl - vo
                    if 0 <= kv <= 2:
                        # lane p = (cin*2+kh)*10+vl ; col m = vo*4+cout
                        wbd[kh * VL + vl : KD : 2 * VL, ri, vo * 4 : vo * 4 + 4] = w[
                            :, :, ku, kv, kh, kw
                        ].T
    bias = np.tile(np.asarray(b, np.float32), 32).reshape(128, 1)
    return wbd.astype(ml_dtypes.bfloat16), bias


def _host_shard(x):
    x = np.asarray(x, np.float32)
    shards = []
    vidx = (8 * np.arange(NB)[:, None] + np.arange(VL)[None, :]).reshape(-1)  # 0..33
    for core in range(NCORES):
        bb, hh = divmod(core, 2)
        xc = x[bb, :, :, :, 32 * hh : 32 * hh + 32, :]  # [cin, r, v, h, w]
        xp = np.pad(xc, ((0, 0), (0, 0), (1, 1), (0, 0), (0, 0)))  # v -> 34
        arr = xp[:, :, vidx]  # [cin, r, b*vl, h, w]
        arr = arr.reshape(CIN, U, NB, VL, IL, 2, J, 2)
        # dims: cin0 r1 b2 vl3 i4 kh5 j6 kw7 -> [r, cin, kh, vl, kw, b, i, j]
        arr = arr.transpose(1, 0, 5, 3, 7, 2, 4, 6)
        arr = np.ascontiguousarray(arr).astype(ml_dtypes.float8_e3m4)
        arr = arr.reshape(U, KD, 2, NB, IL, J)
        fat = np.zeros((NZ, K, 2, NB, IL, J), ml_dtypes.float8_e3m4)
        fat[:, :KD] = arr[:NZ]
        shards.append({"zs0": fat, "zs1": np.ascontiguousarray(arr[NZ:])})
    return shards


def _build_program():
    nc = bacc.Bacc("TRN2", target_bir_lowering=False, debug=False)
    zs0 = nc.dram_tensor("zs0", [NZ, K, 2, NB, IL, J], F8E3, kind="ExternalInput").ap()
    zs1 = nc.dram_tensor("zs1", [U - NZ, KD, 2, NB, IL, J], F8E3, kind="ExternalInput").ap()
    wbd = nc.dram_tensor("wbd", [K, 6, 32], BF16, kind="ExternalInput").ap()
    bias = nc.dram_tensor("bias", [128, 1], F32, kind="ExternalInput").ap()
    ys = nc.dram_tensor("ys", [U, 128, IL, J], F16, kind="ExternalOutput").ap()

    with ExitStack() as ctx:
        zt = [ctx.enter_context(nc.sbuf_tensor(f"z{i}", [K, 2, NB, IL, J], F8E3)) for i in range(NZ)]
        wt = ctx.enter_context(nc.sbuf_tensor("wt", [K, 6, 32], BF16))
        bt = ctx.enter_context(nc.sbuf_tensor("bt", [128, 1], F32))
        ot = [ctx.enter_context(nc.sbuf_tensor(f"ot{i}", [128, IL, J], F16)) for i in range(NOUT)]
        ps = [ctx.enter_context(nc.psum_tensor(f"ps{i}", [128, IL, J], F32)) for i in range(NPS)]
        sem_z = [ctx.enter_context(nc.semaphore(f"sem_z{r}")) for r in range(U)]
        sem_w = ctx.enter_context(nc.semaphore("sem_w"))
        sem_b = ctx.enter_context(nc.semaphore("sem_b"))
        sem_mm = ctx.enter_context(nc.semaphore("sem_mm"))
        sem_act = ctx.enter_context(nc.semaphore("sem_act"))
        sem_os = [ctx.enter_context(nc.semaphore(f"sem_o{i}")) for i in range(NOUT)]
        blk_ctx = nc.Block()
        block = blk_ctx.__enter__()

        def _loader(eng, par):
            # first row immediately; rest staged behind sem_w so the first
            # group's deps aren't diluted on the shared DMA device
            eng.dma_start(zt[par][:], zs0[par]).then_inc(sem_z[par], 16)
            eng.wait_ge(sem_w, 16)
            for r in range(2 + par, NZ, 2):
                eng.dma_start(zt[r][:], zs0[r]).then_inc(sem_z[r], 16)
            for r in range(NZ + par, U, 2):
                eng.wait_ge(sem_mm, r - 6)
                eng.dma_start(zt[r % NZ][:KD], zs1[r - NZ]).then_inc(sem_z[r], 16)

        @block.sync
        def _(sync):
            _loader(sync, 0)
            # last odd stores run here so gpsimd's slow SWDGE drain overlaps
            for u in (29, 31):
                sync.wait_ge(sem_act, u + 1)
                sync.dma_start(ys[u], ot[u % NOUT][:]).then_inc(
                    sem_os[u % NOUT], 16
                )
            for s in sem_os:
                sync.wait_ge(s, 16 * (U // NOUT))

        @block.gpsimd
        def _(gpsimd):
            gpsimd.dma_start(zt[1][:], zs0[1]).then_inc(sem_z[1], 16)
            gpsimd.wait_ge(sem_w, 16)
            for r in range(3, NZ, 2):
                gpsimd.dma_start(zt[r][:], zs0[r]).then_inc(sem_z[r], 16)
            for u in range(1, U, 2):
                if u <= 27:
                    gpsimd.wait_ge(sem_act, u + 1)
                    gpsimd.dma_start(ys[u], ot[u % NOUT][:]).then_inc(
                        sem_os[u % NOUT], 16
                    )
                r = u + NZ
                if r < U:
                    gpsimd.wait_ge(sem_mm, r - 6)
                    gpsimd.dma_start(zt[r % NZ][:KD], zs1[r - NZ]).then_inc(
                        sem_z[r], 16
                    )

        @block.scalar
        def _(scalar):
            scalar.dma_start(wt[:], wbd[:]).then_inc(sem_w, 16)
            scalar.dma_start(bt[:], bias[:]).then_inc(sem_b, 16)
            for u in range(U):
                scalar.wait_ge(sem_mm, u + 1)
                if u == 0:
                    scalar.wait_ge(sem_b, 16)
                if u >= NOUT:
                    scalar.wait_ge(sem_os[u % NOUT], 16 * (u // NOUT))
                nc.scalar.activation(
                    ot[u % NOUT][:],
                    ps[u % NPS][:],
                    mybir.ActivationFunctionType.Identity,
                    bias=bt[:],
                ).then_inc(sem_act)
                if u % 2 == 0:
                    scalar.dma_start(ys[u], ot[u % NOUT][:]).then_inc(
                        sem_os[u % NOUT], 16
                    )

        @block.tensor
        def _(tensor):
            for u in range(U):
                if u == 0:
                    tensor.wait_ge(sem_w, 16)
                    tensor.wait_ge(sem_z[0], 16)
                    tensor.wait_ge(sem_z[1], 16)
                elif u + 1 < U:
                    tensor.wait_ge(sem_z[u + 1], 16)
                if u >= NPS:
                    tensor.wait_ge(sem_act, u - NPS + 1)
                psg = ps[u % NPS]
                rounds = [(ku, kw) for ku in range(3) if 0 <= u + ku - 1 < U for kw in range(2)]
                last = None
                for idx, (ku, kw) in enumerate(rounds):
                    ri = ku * 2 + kw
                    slot = (u + ku - 1) % NZ
                    for b4 in range(NB):
                        last = nc.tensor.matmul(
                            psg[32 * b4 : 32 * b4 + 32, :, :],
                            wt[:, ri, :],
                            zt[slot][:, kw, b4, :, :],
                            start=(idx == 0),
                            stop=(idx == len(rounds) - 1),
                            skip_group_check=True,
                            tile_position=(0, 32 * b4),
                        )
                last.then_inc(sem_mm)

        blk_ctx.__exit__(None, None, None)

    nc.compile()
    return nc


def _unshard(results):
    y = np.empty((B, COUT, D1, D2, H // 2, W // 2), np.float32)
    for core in range(NCORES):
        bb, hh = divmod(core, 2)
        arr = results[core]["ys"].astype(np.float32)
        arr = arr.reshape(U, NB, 8, COUT, IL, J)
        arr = arr.transpose(3, 0, 1, 2, 4, 5).reshape(COUT, U, V, IL, J)
        y[bb, :, :, :, 16 * hh : 16 * hh + 16, :] = arr
    return y


TRACE = False
LAST_RESULT = [None]

def kernel(x, w, b, _cache={}):
    if "nc" not in _cache:
        _cache["nc"] = _build_program()
    nc = _cache["nc"]
    wbd, bias = _host_weights(w, b)
    in_maps = [dict(sh, wbd=wbd, bias=bias) for sh in _host_shard(x)]
    res = run_bass_kernel_spmd(nc, in_maps, list(range(NCORES)), trace=TRACE)
    LAST_RESULT[0] = res
    return _unshard(res.results)
